# revision 18
# baseline (speedup 1.0000x reference)
import sys

sys.path.insert(0, "/opt/trn_rl_repo")

import numpy as np

import concourse.bacc as bacc
import concourse.bass as bass
import concourse.mybir as mybir
import concourse.tile as tile
from concourse.bass_utils import run_bass_kernel_spmd

# Problem shapes (hardcoded per contract)
B = 4
NQ = 2048
NR = 16384
D = 64
K = 16

NCORES = 8
QPC = NQ // 2          # queries per core (each batch split across 2 cores)
NCHUNK = QPC // 128    # query chunks of 128 per core
MMN = 512              # matmul free dim (one PSUM bank of fp32)
PAIR = 2048            # refs per staging tile (4 PSUM banks); top-8 per block
NPAIR = NR // PAIR     # 8
NSLOT = 4              # staging slots (Act->DVE pipeline depth)
NCAND = NPAIR * 8      # 64 candidates per row

_prog_cache = {}


def _build_program(reps: int = 1):
    if reps in _prog_cache:
        return _prog_cache[reps]

    f32 = mybir.dt.float32
    f32r = mybir.dt.float32r
    f16 = mybir.dt.float16
    u32 = mybir.dt.uint32

    nc = bacc.Bacc("TRN2", target_bir_lowering=False, debug=False, num_devices=NCORES)

    # lhsT rows 0..63 = 2*q^T, row 64 = 1.0, row 65 = q2  -> psum = 2qr - r2 - q2 = -d2
    lhs_d = nc.dram_tensor("lhs", [66, QPC], f32r, kind="ExternalInput")
    rhs_d = nc.dram_tensor("rhs", [66, NR], f32r, kind="ExternalInput")

    # 64 candidate composites per query: fp16(-d2) in high 16 bits, local ref
    # idx in low 11; candidate column s comes from ref block s >> 3
    outC_d = nc.dram_tensor("outC", [QPC, NCAND], u32, kind="ExternalOutput")

    with tile.TileContext(nc) as tc:
        with (
            tc.tile_pool(name="consts", bufs=1) as cpool,
            tc.tile_pool(name="psum", bufs=2, space="PSUM") as ppool,
            tc.tile_pool(name="merge", bufs=2) as mpool,
        ):
            lhs_t = cpool.tile([66, QPC], f32r)
            rhs_t = cpool.tile([66, NR], f32r)
            # pair-0 / chunk-0 operands first so compute starts ASAP
            nc.sync.dma_start(lhs_t[:, 0:128], lhs_d.ap()[:, 0:128])
            nc.sync.dma_start(rhs_t[:, 0:MMN], rhs_d.ap()[:, 0:MMN])
            nc.sync.dma_start(lhs_t[:, 128:QPC], lhs_d.ap()[:, 128:QPC])
            nc.sync.dma_start(rhs_t[:, MMN:PAIR], rhs_d.ap()[:, MMN:PAIR])
            for p in range(1, NPAIR):
                c0, c1 = p * PAIR, (p + 1) * PAIR
                nc.sync.dma_start(rhs_t[:, c0:c1], rhs_d.ap()[:, c0:c1])

            # trigger the activation-table load before real work
            actwarm = cpool.tile([128, 1], f32)
            nc.gpsimd.memset(actwarm[:], 0.0)
            nc.scalar.activation(
                actwarm[:], actwarm[:], mybir.ActivationFunctionType.Copy
            )

            # composite staging slots; low halves = local ref idx (0..PAIR-1),
            # written once by the otherwise-idle gpsimd engine
            stages = []
            for s in range(NSLOT):
                st = cpool.tile([128, PAIR], f32, name=f"stage{s}")
                nc.gpsimd.iota(
                    st.bitcast(u32)[:], pattern=[[1, PAIR]], base=0,
                    channel_multiplier=0,
                )
                stages.append(st)

            for rep in range(reps):
              for c in range(NCHUNK):
                cands = mpool.tile([128, NCAND], u32, tag="cands", bufs=2)
                for p in range(NPAIR):
                    ps = ppool.tile([128, PAIR], f32, tag="ps")
                    for h in range(PAIR // MMN):
                        nc.tensor.matmul(
                            ps[:, h * MMN:(h + 1) * MMN],
                            lhs_t[:, c * 128:(c + 1) * 128],
                            rhs_t[:, p * PAIR + h * MMN:p * PAIR + (h + 1) * MMN],
                            start=True,
                            stop=True,
                        )
                    # -d2 as fp16 into composite high halves (strided write)
                    st = stages[p % NSLOT]
                    nc.scalar.activation(
                        st.bitcast(f16)[:, 1::2],
                        ps[:],
                        mybir.ActivationFunctionType.Copy,
                    )
                    s = p * 8
                    nc.vector.max(cands.bitcast(f32)[:, s:s + 8], st[:])

                r0, r1 = c * 128, (c + 1) * 128
                if c == NCHUNK - 1:
                    # split the final output so the tail DMA is tiny
                    nc.sync.dma_start(
                        outC_d.ap()[r0:r1, 0:NCAND // 2], cands[:, 0:NCAND // 2]
                    )
                    nc.sync.dma_start(
                        outC_d.ap()[r0:r1, NCAND // 2:], cands[:, NCAND // 2:]
                    )
                else:
                    nc.sync.dma_start(outC_d.ap()[r0:r1, :], cands[:])

    nc.compile()
    _prog_cache[reps] = nc
    return nc


def kernel(ref: np.ndarray, query: np.ndarray):
    ref = np.asarray(ref, dtype=np.float32)
    query = np.asarray(query, dtype=np.float32)

    # host-side operand prep (layout + norms)
    r2 = np.sum(ref * ref, axis=-1)                      # [B, NR]
    q2 = np.sum(query * query, axis=-1)                  # [B, NQ]
    refT = np.ascontiguousarray(ref.transpose(0, 2, 1))  # [B, D, NR]
    qT = np.ascontiguousarray(query.transpose(0, 2, 1))  # [B, D, NQ]

    nc = _build_program()

    in_maps = []
    for core in range(NCORES):
        b, h = core // 2, core % 2
        lhs = np.empty((66, QPC), dtype=np.float32)
        lhs[0:D, :] = 2.0 * qT[b][:, h * QPC:(h + 1) * QPC]
        lhs[D, :] = 1.0
        lhs[D + 1, :] = q2[b, h * QPC:(h + 1) * QPC]
        rhs = np.empty((66, NR), dtype=np.float32)
        rhs[0:D, :] = refT[b]
        rhs[D, :] = -r2[b]
        rhs[D + 1, :] = -1.0
        in_maps.append({"lhs": lhs, "rhs": rhs})

    res = run_bass_kernel_spmd(nc, in_maps, core_ids=list(range(NCORES)))

    # candidate column s -> ref block s >> 3
    base = ((np.arange(NCAND) >> 3) * PAIR).astype(np.int64)[None, :]
    rows = np.arange(QPC)[:, None]
    Dout = np.empty((B, NQ, K), dtype=np.float32)
    Iout = np.empty((B, NQ, K), dtype=np.int64)
    for core in range(NCORES):
        b, h = core // 2, core % 2
        comp = res.results[core]["outC"].astype(np.uint32)   # [QPC, NCAND]
        gidx = base + (comp & 0x7FF).astype(np.int64)        # global ref idx
        # merge: top-16 of 64 by composite order (desc composite = asc d2)
        top = np.argsort(comp.view(np.float32), axis=1, kind="stable")[:, :-K - 1:-1]
        idx = gidx[rows, top]                                # [QPC, K]
        # exact rescore of the 16 selected candidates (fixes quantization-
        # induced order swaps among near-ties)
        qs = query[b, h * QPC:(h + 1) * QPC]                 # [QPC, D]
        cand = ref[b][idx]                                   # [QPC, K, D]
        d2 = np.maximum(0.0, np.sum((cand - qs[:, None, :]) ** 2, axis=-1))
        perm = np.lexsort((idx, d2), axis=1)
        Dout[b, h * QPC:(h + 1) * QPC] = np.sqrt(d2[rows, perm])
        Iout[b, h * QPC:(h + 1) * QPC] = idx[rows, perm]
    return (Dout, Iout)
